# revision 21
# baseline (speedup 1.0000x reference)
"""Masked-softmax cross-entropy loss on 8 Trainium2 cores.

Math per target row t (16384 rows of length 4096):
  numer[t] = sum_j exp(x[t,j]/tau) over valid src cols j with color == tgt color t
  denom[t] = sum_j exp(x[t,j]/tau) over valid src cols j
  p_gt = numer/denom, nll = -log(p_gt + eps); rows with numer==0 are masked.
Segment aggregation (32 segments) happens on host - it touches 16K scalars.

Sharding: core c takes half a batch: batch c//2, row-half c%2 (2048 rows).

Device design (v2) - transposed layout + TensorE color buckets:
  The similarity map is shipped as f16 *transposed*: per core x^T is
  [4096 src j, 2048 tgt t], reshaped on host so chunk ci is a [128, 4096]
  DRAM slab-pair (partition = j within slab, free = slab-half * 2048 + t).
  Per chunk: SWDGE 1MB load -> ScalarE in-place exp(10*x) (f16) -> PE
  matmuls against a per-slab one-hot color matrix W [128 j, 99]:
     W[j, c] = 1 if src color id of j == c (c in 0..97), and
     W[j, 98] = 1 if j is a valid (non-pad) src column.
  PSUM accumulates bucket[c, t] over all 32 slabs; column 98 is the valid
  denominator. One DVE copy PSUM->SBUF and one output DMA per core.
  Host gathers numer[t] = bucket[tid[t], t], denom[t] = bucket[98, t].
  This keeps ScalarE (the exp engine, 1 elem/cycle/lane) as the only
  saturated engine: ~59us of exp vs ~142us of 1x-mode DVE STTs in v1.

Sync-wait budget: walrus allows very few sem waits per instruction
(1 for DMA/CTRL). Tiny pool-queue "interposer" copies absorb the
extra cross-engine waits ahead of each load; the kernel-tail drain is
split into one drain per proc.
"""

import os
import numpy as np

B = 4
S_TGT = 8
L_TGT = 512
C = 4
N = 4096          # src columns (= 8*512), also total tgt rows per batch
P = 128
ROWS = 2048       # tgt rows per core (half a batch)
NSLAB = N // P    # 32 j-slabs of 128 src columns
NCHUNK = NSLAB // 2   # 16 chunks; chunk = 2 slabs = [128, 4096] f16 = 1MB
NBUF = 4          # chunk buffer depth (slot reuse distance)
NID = 98          # color ids occupy 0..97 (97 palette colors + pad color)
MCOL = NID + 1    # one-hot columns: 98 id buckets + 1 valid-mask denom col
QCHUNK = 4        # PSUM t-chunks of 512 (one bank each)
NCORES = 8
PAD = -1.0
EPS = 1e-15

_NC_CACHE = {}


def _patch_split_drain():
    """Split the kernel-tail drain's sem waits across several drain
    instructions (walrus rejects >1 sync wait on one CTRL instruction)."""
    import concourse.tile as tile
    from concourse.vector_clock import ScopedClock, VectorClock

    if getattr(tile.TileContext, "_split_drain_patched", False):
        return

    def _drain_and_barrier(self, tick_clock, wait_clock):
        g = tick_clock.global_clock
        n = len(g)
        for base in range(n):
            vec = [g[i] if i == base else 0 for i in range(n)]
            if not any(vec):
                continue
            d = self.nc.sync.drain()
            wait_clock.add_sem_waits(d.ins, ScopedClock({None: VectorClock(vec)}))
        self.nc.all_engine_barrier()
        popped = self.nc._tile_sem_poison_stack.pop()
        assert popped is self._sem_poison
        self.nc.clear_and_free_semaphores(list(self.sems.allocated().values()))
        self.nc.all_engine_barrier()

    tile.TileContext._drain_and_barrier = _drain_and_barrier
    tile.TileContext._split_drain_patched = True


def _build_nc():
    import concourse.bass as bass
    import concourse.mybir as mybir
    import concourse.tile as tile
    from concourse.tile_rust import add_dep_helper
    from contextlib import ExitStack

    _patch_split_drain()
    nc = bass.Bass()
    f32 = mybir.dt.float32
    f16 = mybir.dt.float16
    bf16 = mybir.dt.bfloat16
    NW = 2 * N  # unused width marker (chunk free size is 4096)

    # x chunk layout: row 128*ci + p, col 2048*hh + t  <=>  x^T[j, t] with
    # j = 128*(2*ci + hh) + p  (host packs it this way)
    x = nc.declare_dram_parameter("x", [NCHUNK * P, 2 * ROWS], f16, isOutput=False)
    # w layout: [p, 99*s + c] = one-hot for src col j = 128*s + p
    w = nc.declare_dram_parameter("w", [P, NSLAB * MCOL], bf16, isOutput=False)
    buckets = nc.declare_dram_parameter("buckets", [MCOL, ROWS], bf16,
                                        isOutput=True)

    with tile.TileContext(nc) as tc:
        with ExitStack() as ctx:
            const_pool = ctx.enter_context(tc.tile_pool(name="const", bufs=1))
            # x input tiles (f16, recycled): the recycle waits are absorbed
            # ahead of each load. et output tiles (bf16 - exp(10x) reaches
            # 7e23, far over f16 max, and the numerator needs range down to
            # ~e^-55) are never recycled: 16 x 8KB/partition = 128KB, so
            # each exp writes fresh SBUF and carries only its load's wait.
            x_pool = ctx.enter_context(tc.tile_pool(name="x", bufs=NBUF))
            data_pool = ctx.enter_context(tc.tile_pool(name="data", bufs=1))
            res_pool = ctx.enter_context(tc.tile_pool(name="res", bufs=1))
            psum_pool = ctx.enter_context(
                tc.tile_pool(name="psum", bufs=1, space="PSUM")
            )

            wt = const_pool.tile([P, NSLAB * MCOL], bf16)

            # absorber scratch: one column per (chunk, kind)
            scr = const_pool.tile([P, 3 * NCHUNK + 4], f16)
            warm = const_pool.tile([P, 2], f16)
            junk = const_pool.tile([P, 1], f16)

            res_a = res_pool.tile([P, ROWS // 2], bf16)
            res_b = res_pool.tile([P, ROWS // 2], bf16)
            # two PSUM tiles (2 banks each) so the tail copies on ACT and
            # DVE don't share a tile (shared-tile deps serialize them)
            pt_a = psum_pool.tile([P, ROWS // 2], f32)
            pt_b = psum_pool.tile([P, ROWS // 2], f32)

            # warm-ups read a Pool-memset tile, NOT wt: a wt read would make
            # the Pool warm-up (and with it the whole Pool FIFO, i.e. every
            # x-load issue) block on the w const-DMA. The first junk reader
            # per engine absorbs the Pool-clock wait; later readers elide it.
            # The ACT warm-up doubles as the exp table-load trigger.
            nc.gpsimd.memset(junk[:], 0.0)
            nc.gpsimd.tensor_copy(warm[:, 1:2], junk[:])
            nc.scalar.activation(
                warm[:, 0:1], junk[:], mybir.ActivationFunctionType.Exp,
                scale=10.0,
            )
            load_insts = []   # one entry per dma_start, in issue order
            chunk_loads = []  # per chunk: list of its dma_start insts
            mm_last = []      # last matmul per chunk
            nscr = [0]

            def absorber(dep_inst, reason):
                # pool-queue interposer: carries one foreign sem wait so the
                # following DMACopy keeps at most one sync wait of its own
                a = nc.gpsimd.tensor_copy(
                    scr[:, nscr[0]:nscr[0] + 1], junk[:]
                )
                nscr[0] += 1
                add_dep_helper(a.ins, dep_inst.ins, sync=True, reason=reason)
                return a

            for ci in range(NCHUNK):
                split = ci <= 1 or ci == NCHUNK - 1
                xt = x_pool.tile([P, 2 * ROWS], f16)
                lds = []
                nld = 2 if split else 1
                for li in range(nld):
                    pre = []
                    if ci >= NBUF:
                        # x-slot WAW vs the old chunk's load(s)
                        for old in chunk_loads[ci - NBUF]:
                            pre.append(absorber(old, "absorb x-slot WAW"))
                    k = len(load_insts)
                    if k >= 8:
                        pre.append(absorber(load_insts[k - 8],
                                            "absorb DMA lane WAW"))
                    if split:
                        # chunk 0: two 512KB halves so the first exp starts
                        # ~2.5us earlier; chunk 15: same split so the last
                        # MMs overlap the second half-exp (shorter tail)
                        ld = nc.gpsimd.dma_start(
                            xt[:, li * ROWS:(li + 1) * ROWS],
                            x[ci * P:(ci + 1) * P, li * ROWS:(li + 1) * ROWS],
                        )
                    else:
                        ld = nc.gpsimd.dma_start(
                            xt[:], x[ci * P:(ci + 1) * P, :]
                        )
                    for g in pre:
                        add_dep_helper(
                            ld.ins, g.ins, sync=False,
                            reason="load ordered after wait absorber",
                        )
                    load_insts.append(ld)
                    lds.append(ld)
                chunk_loads.append(lds)
                if ci == 0:
                    # issue the w const-DMA only after chunk 0 has landed:
                    # it would otherwise share the SDMA engines with the
                    # first x load and delay the first exp by ~3us. The SP
                    # queue is idle until the tail, so the wait is free.
                    wld = nc.sync.dma_start(wt[:], w[:])
                    add_dep_helper(wld.ins, lds[1].ins, sync=True,
                                   reason="w load after first x chunk")
                    # PE absorber for the w const-DMA wait (only PE reads wt)
                    nc.tensor.ldweights(wt[:, 0:MCOL])

                # exp into a fresh bf16 tile: its only sem wait is the DMA
                et = data_pool.tile([P, 2 * ROWS], bf16, name=f"et{ci}",
                                    tag=f"et{ci}")
                last = None
                for hh in range(2):
                    if split or hh == 0:
                        e = nc.scalar.activation(
                            et[:, hh * ROWS:(hh + 1) * ROWS] if split else et[:],
                            xt[:, hh * ROWS:(hh + 1) * ROWS] if split else xt[:],
                            mybir.ActivationFunctionType.Exp, scale=10.0,
                        )
                    s = 2 * ci + hh
                    for q in range(QCHUNK):
                        ptile = pt_a if q < 2 else pt_b
                        last = nc.tensor.matmul(
                            ptile[0:MCOL, (q % 2) * 512:(q % 2 + 1) * 512],
                            wt[:, s * MCOL:(s + 1) * MCOL],
                            et[:, hh * ROWS + q * 512:hh * ROWS + (q + 1) * 512],
                            start=(s == 0),
                            stop=(s == NSLAB - 1),
                            skip_group_check=True,
                        )
                mm_last.append(last)

            # tail: split the PSUM->SBUF copy across ACT and DVE (they run
            # in parallel; separate res tiles avoid a cross-engine tile-WAW
            # wait), each half feeding its own output DMA
            nc.scalar.copy(res_a[0:MCOL, :], pt_a[0:MCOL, :])
            nc.vector.tensor_copy(res_b[0:MCOL, :], pt_b[0:MCOL, :])
            # two HWDGE rings (ACT's and SP's) so the two output transfers
            # overlap instead of serializing on one ring
            nc.scalar.dma_start(buckets[:, 0:ROWS // 2], res_a[0:MCOL, :])
            nc.sync.dma_start(buckets[:, ROWS // 2:], res_b[0:MCOL, :])
    return nc


def _get_nc():
    key = (NBUF, NCHUNK)
    if key not in _NC_CACHE:
        _NC_CACHE[key] = _build_nc()
    return _NC_CACHE[key]


def _color_ids(src, tgt):
    """Map each color row to a per-batch integer id via exact byte equality."""
    src_f = np.ascontiguousarray(src.reshape(B, -1, C))
    tgt_f = np.ascontiguousarray(tgt.reshape(B, -1, C))
    n_s = src_f.shape[1]
    src_ids = np.empty((B, n_s), np.int32)
    tgt_ids = np.empty((B, tgt_f.shape[1]), np.int32)
    for b in range(B):
        allc = np.ascontiguousarray(np.concatenate([src_f[b], tgt_f[b]], axis=0))
        view = allc.view([("", allc.dtype)] * C).reshape(-1)
        _, inv = np.unique(view, return_inverse=True)
        ids = inv.astype(np.int32)
        s_ids, t_ids = ids[:n_s].copy(), ids[n_s:].copy()
        s_ids[np.all(src_f[b] == PAD, axis=-1)] = -1
        t_ids[np.all(tgt_f[b] == PAD, axis=-1)] = -2
        src_ids[b], tgt_ids[b] = s_ids, t_ids
    return src_ids, tgt_ids


def kernel(seg_sim_map, seg_colors_src, seg_colors_tgt):
    import ml_dtypes
    from concourse.bass_utils import run_bass_kernel_spmd

    seg_sim_map = np.asarray(seg_sim_map, dtype=np.float32)
    src_ids, tgt_ids = _color_ids(
        np.asarray(seg_colors_src, np.float32), np.asarray(seg_colors_tgt, np.float32)
    )
    assert src_ids.max() < NID and tgt_ids.max() < NID

    # per-batch one-hot W in the device layout [P, 32*99]
    w_dev = {}
    for b in range(B):
        onehot = np.zeros((N, MCOL), ml_dtypes.bfloat16)
        valid = src_ids[b] >= 0
        onehot[np.arange(N)[valid], src_ids[b][valid]] = 1.0
        onehot[valid, NID] = 1.0  # denom column: any valid src col
        # [N, MCOL] -> [NSLAB, P, MCOL] -> [P, NSLAB, MCOL] -> [P, NSLAB*MCOL]
        w_dev[b] = np.ascontiguousarray(
            onehot.reshape(NSLAB, P, MCOL).transpose(1, 0, 2).reshape(P, -1)
        )

    in_maps = []
    for c in range(NCORES):
        b, h = c // 2, c % 2
        rows = slice(h * ROWS, (h + 1) * ROWS)
        # x^T: [4096 j, 2048 t] -> chunks [16, 2, 128, 2048] ->
        # [16, 128, 2, 2048] -> [2048, 4096]
        xT = np.ascontiguousarray(seg_sim_map[b, rows, :].T.astype(np.float16))
        x_dev = np.ascontiguousarray(
            xT.reshape(NCHUNK, 2, P, ROWS).transpose(0, 2, 1, 3).reshape(
                NCHUNK * P, 2 * ROWS
            )
        )
        in_maps.append({"x": x_dev, "w": w_dev[b]})

    trace = os.environ.get("KERNEL_PROFILE", "") == "1"
    nc = _get_nc()
    out = run_bass_kernel_spmd(nc, in_maps, list(range(NCORES)), trace=trace)
    if trace and out.exec_time_ns is not None:
        print(f"HW exec time: {out.exec_time_ns} ns")
        print(f"HW exec mean: {out.mean_exec_time_ns} ns")

    numer = np.empty((B, N), np.float32)
    denom = np.empty((B, N), np.float32)
    for c in range(NCORES):
        b, h = c // 2, c % 2
        rows = slice(h * ROWS, (h + 1) * ROWS)
        buckets = np.asarray(out.results[c]["buckets"], np.float32)  # [99, 2048]
        tid = tgt_ids[b, rows]
        gather = buckets[np.where(tid >= 0, tid, 0), np.arange(ROWS)]
        numer[b, rows] = np.where(tid >= 0, gather, 0.0)
        denom[b, rows] = buckets[NID]

    # host finalize, mirroring the reference ops in f32 (touches 16K scalars)
    p_gt = numer / denom
    nll = -np.log(p_gt + np.float32(EPS))
    m = (numer > 0).astype(np.float32)
    nll3 = nll.reshape(B, S_TGT, L_TGT)
    m3 = m.reshape(B, S_TGT, L_TGT)
    nvalid = m3.sum(-1)
    seg_loss = np.where(
        nvalid > 0, (nll3 * m3).sum(-1) / np.maximum(nvalid, np.float32(1.0)), 0.0
    ).astype(np.float32)
    cnt = int((nvalid > 0).sum())
    total = np.float32(seg_loss.sum(dtype=np.float32) / np.float32(max(cnt, 1)))
    return np.asarray(total, np.float32), np.asarray(cnt, np.int32)
